# revision 15
# baseline (speedup 1.0000x reference)
"""Bidirectional-LSTM basecaller kernel for 8 Trainium2 NeuronCores (Bass/Tile).

Model: conv front-end (1x1 -> 256, relu; 3x1 256->256, relu; 1x1 256->256,
relu; + 1x1 skip w/ bias) -> LSTM(H=200) per direction -> dense 2H->5.
B=32, T=2048.

Sharding: 4 batch groups x 2 directions = 8 cores, 8 sequences per core,
same SPMD program everywhere. The backward direction is realized host-side
by left-aligned per-sequence time reversal with a zero tail plus flipped
conv kernels, which makes the on-device scan completely mask-free; the host
un-reverses the tiny per-core partial logits and applies the length mask.

On-device layout (per core):
  - conv runs in channel-transposed layout [256ch -> 2x128 partitions,
    (t,b) free, t-major], K=3 conv = 6 accumulating matmuls with +-8-column
    shifted moving operands.
  - z_x = enc @ Wx + b is precomputed into PSUM per 32-step chunk (4 banks
    per parity, ping-pong); bias enters via a K=1 matmul against ones.
  - scan step: 16 matmuls (8 gate-tiles of 100 x 2 K-halves of 100, Wh
    stationary bf16) accumulate the recurrent term onto z_x in PSUM;
    sigmoid/tanh run on [100,2,8] PSUM slices; state c/h kept [100,(2,8)]
    so the new h is directly the next step's moving operand. No transposes.
  - decode tail: h_seq @ Wd_half -> partial logits [5, T*8] fp32 -> HBM.
"""
import sys
import numpy as np

sys.path.insert(0, "/opt/trn_rl_repo")

import ml_dtypes

BF16 = ml_dtypes.bfloat16

B, T_FULL, H, C = 32, 2048, 200, 256
NSEQ = 8          # sequences per core
HH = 100          # half hidden
G4 = 4 * H        # 800
N_CORES = 8
SCAN_CHUNK = 32   # timesteps per PSUM parity chunk


def build_program(T=T_FULL, conv_chunk=256, debug_z=False):
    """Emit the SPMD per-core program. Returns finalized Bacc object."""
    import concourse.bass as bass
    import concourse.tile as tile
    from concourse import bacc, mybir

    dt = mybir.dt
    BF = dt.bfloat16
    F32 = dt.float32
    AF = mybir.ActivationFunctionType

    assert T % (2 * SCAN_CHUNK) == 0
    N = T * NSEQ
    CC = min(conv_chunk, T)          # conv chunk timesteps
    NCC = CC * NSEQ                  # conv chunk cols
    n_conv_chunks = T // CC
    n_chunks = T // SCAN_CHUNK       # scan chunks

    nc = bacc.Bacc()

    # cols 0:8 = t=-1 halo, 8:8+N = data, 8+N:16+N = t=T halo (host-filled)
    sig_d = nc.declare_dram_parameter("sig", [1, N + 16], BF, isOutput=False)
    k1w_d = nc.declare_dram_parameter("k1w", [1, C], BF, isOutput=False)
    k1aw_d = nc.declare_dram_parameter("k1aw", [1, C], BF, isOutput=False)
    k1ab_d = nc.declare_dram_parameter("k1ab", [128, 2], F32, isOutput=False)
    k2w_d = nc.declare_dram_parameter("k2w", [128, 12 * 128], BF, isOutput=False)
    k3w_d = nc.declare_dram_parameter("k3w", [128, 4 * 128], BF, isOutput=False)
    wx_d = nc.declare_dram_parameter("wx", [128, 16 * 128], BF, isOutput=False)
    whr_d = nc.declare_dram_parameter("whr", [HH, 16 * 128], BF, isOutput=False)
    bz_d = nc.declare_dram_parameter("bz", [1, 8 * 128], BF, isOutput=False)
    wd_d = nc.declare_dram_parameter("wd", [HH, 16], BF, isOutput=False)
    out_d = nc.declare_dram_parameter("logits", [5, N], F32, isOutput=True)
    zdbg_d = None
    if debug_z:
        zdbg_d = nc.declare_dram_parameter("zdbg", [128, 4 * 512], F32, isOutput=True)

    with tile.TileContext(nc) as tc:
        with (
            tc.tile_pool(name="wpool", bufs=1) as wpool,
            tc.tile_pool(name="state", bufs=1) as spool,
        ):
            # ---- persistent tiles ----
            k1w_s = wpool.tile([1, C], BF, tag="k1w")
            k1aw_s = wpool.tile([1, C], BF, tag="k1aw")
            k1ab_s = wpool.tile([128, 2], F32, tag="k1ab")
            k2w_s = wpool.tile([128, 12 * 128], BF, tag="k2w")
            k3w_s = wpool.tile([128, 4 * 128], BF, tag="k3w")
            wx_s = wpool.tile([128, 16 * 128], BF, tag="wx")
            whr_s = wpool.tile([HH, 16 * 128], BF, tag="whr")
            bz_s = wpool.tile([1, 8 * 128], BF, tag="bz")
            wd_s = wpool.tile([HH, 16], BF, tag="wd")
            ones_s = wpool.tile([1, 256], BF, tag="ones")
            encT = [wpool.tile([128, N], BF, tag=f"encT{mc}", name=f"encT{mc}") for mc in range(2)]
            h_seq = wpool.tile([HH, T, 2, NSEQ], BF, tag="h_seq")
            h0_s = spool.tile([HH, 2, NSEQ], BF, tag="h0")
            c_s = spool.tile([HH, 2, NSEQ], F32, tag="c")

            nc.sync.dma_start(k1w_s[:], k1w_d[:])
            nc.sync.dma_start(k1aw_s[:], k1aw_d[:])
            nc.sync.dma_start(k1ab_s[:], k1ab_d[:])
            nc.sync.dma_start(k2w_s[:], k2w_d[:])
            nc.sync.dma_start(k3w_s[:], k3w_d[:])
            nc.sync.dma_start(wx_s[:], wx_d[:])
            nc.sync.dma_start(whr_s[:], whr_d[:])
            nc.sync.dma_start(bz_s[:], bz_d[:])
            nc.sync.dma_start(wd_s[:], wd_d[:])
            nc.vector.memset(ones_s[:], 1.0)
            nc.vector.memset(h0_s[:], 0.0)
            nc.vector.memset(c_s[:], 0.0)

            # ================= conv front-end =================
            def sub_ranges(total, step=512):
                off = 0
                while off < total:
                    w = min(step, total - off)
                    yield off, w
                    off += w

            with (
                tc.tile_pool(name="convsb", bufs=2) as cvp,
                tc.tile_pool(name="convps", bufs=4, space="PSUM") as cps,
            ):
                for cc in range(n_conv_chunks):
                    n0 = cc * NCC          # global col base of this chunk
                    # sig chunk with +-8 halo (halo cols come from the input)
                    sgt = cvp.tile([1, NCC + 16], BF, tag="sigc")
                    nc.sync.dma_start(sgt[:], sig_d[0:1, n0:n0 + NCC + 16])

                    c1t = [cvp.tile([128, NCC + 16], BF, tag=f"c1t{mc}", name=f"c1t{mc}") for mc in range(2)]
                    c2t = [cvp.tile([128, NCC], BF, tag=f"c2t{mc}", name=f"c2t{mc}") for mc in range(2)]

                    # conv1 (with halo cols) and conv1a (chunk cols only)
                    for mc in range(2):
                        for off, w in sub_ranges(NCC + 16):
                            ps = cps.tile([128, 512], F32, tag="cps")
                            nc.tensor.matmul(ps[:, 0:w], k1w_s[0:1, mc * 128:(mc + 1) * 128],
                                             sgt[0:1, off:off + w], start=True, stop=True)
                            nc.scalar.activation(c1t[mc][:, off:off + w], ps[:, 0:w], AF.Relu)
                    for mc in range(2):
                        for off, w in sub_ranges(NCC):
                            ps = cps.tile([128, 512], F32, tag="cps")
                            nc.tensor.matmul(ps[:, 0:w], k1aw_s[0:1, mc * 128:(mc + 1) * 128],
                                             sgt[0:1, 8 + off:8 + off + w], start=True, stop=True)
                            nc.scalar.activation(encT[mc][:, n0 + off:n0 + off + w], ps[:, 0:w],
                                                 AF.Relu, bias=k1ab_s[:, mc:mc + 1])
                    # conv2: 3 shifts x 2 cin tiles accumulated
                    for mc in range(2):
                        for off, w in sub_ranges(NCC):
                            ps = cps.tile([128, 512], F32, tag="cps")
                            first = True
                            for s in range(3):
                                for ki in range(2):
                                    idx = (s * 2 + ki) * 2 + mc
                                    nc.tensor.matmul(
                                        ps[:, 0:w],
                                        k2w_s[:, idx * 128:(idx + 1) * 128],
                                        c1t[ki][:, off + s * 8:off + s * 8 + w],
                                        start=first, stop=(s == 2 and ki == 1))
                                    first = False
                            nc.scalar.activation(c2t[mc][:, off:off + w], ps[:, 0:w], AF.Relu)
                    # conv3 + skip add into encT
                    for mc in range(2):
                        for off, w in sub_ranges(NCC):
                            ps = cps.tile([128, 512], F32, tag="cps")
                            for ki in range(2):
                                idx = ki * 2 + mc
                                nc.tensor.matmul(ps[:, 0:w], k3w_s[:, idx * 128:(idx + 1) * 128],
                                                 c2t[ki][:, off:off + w],
                                                 start=(ki == 0), stop=(ki == 1))
                            tmp = cvp.tile([128, 512], BF, tag="tmp")
                            nc.scalar.activation(tmp[:, 0:w], ps[:, 0:w], AF.Relu)
                            nc.vector.tensor_add(encT[mc][:, n0 + off:n0 + off + w],
                                                 tmp[:, 0:w], encT[mc][:, n0 + off:n0 + off + w])

            # ================= LSTM scan =================
            with (
                tc.tile_pool(name="zps", bufs=1, space="PSUM") as zpool,
                tc.tile_pool(name="gates", bufs=2) as gpool,
            ):
                # 8 PSUM banks: [parity][bankj]; bank j holds gate-tiles 2j,2j+1
                # in column halves [0:256) / [256:512).
                zb = [[zpool.tile([128, 512], F32, tag=f"z{p}{j}", name=f"z{p}{j}") for j in range(4)]
                      for p in range(2)]

                def emit_zx_mm(c, i):
                    """i-th of the 24 z_x matmuls for scan chunk c.

                    PSUM start=True zeroes the WHOLE bank (2KB granularity), so
                    only the first matmul touching each bank per chunk sets it;
                    the odd half's first write inherits overwrite-on-first-touch
                    from the pending-zero bytes.
                    """
                    p = c & 1
                    g, which = divmod(i, 3)
                    j, half = divmod(g, 2)
                    out = zb[p][j][:, half * 256:half * 256 + 256]
                    if which < 2:
                        ki = which
                        nc.tensor.matmul(
                            out, wx_s[:, (ki * 8 + g) * 128:(ki * 8 + g + 1) * 128],
                            encT[ki][:, c * 256:(c + 1) * 256],
                            start=(which == 0 and half == 0), stop=False,
                            skip_group_check=True)
                    else:
                        nc.tensor.matmul(
                            out, bz_s[0:1, g * 128:(g + 1) * 128], ones_s[0:1, 0:256],
                            start=False, stop=False, skip_group_check=True)

                def emit_step(c, tl, zx_extra):
                    p = c & 1
                    t = c * SCAN_CHUNK + tl
                    if t == 0:
                        hprev = h0_s
                    else:
                        hprev = h_seq[:, t - 1]
                    # recurrent matmuls; gate order f,i,j,o so sf is ready early
                    for g_pair, gates in ((2, "f"), (0, "i"), (1, "j"), (3, "o")):
                        for half in range(2):
                            g = g_pair * 2 + half
                            j = g // 2
                            out = zb[p][j][:, (g % 2) * 256 + tl * 8:(g % 2) * 256 + tl * 8 + 8]
                            for ki in range(2):
                                nc.tensor.matmul(
                                    out,
                                    whr_s[:, (ki * 8 + g) * 128:(ki * 8 + g + 1) * 128],
                                    hprev[:, ki],
                                    start=False, stop=(ki == 1), skip_group_check=True)
                        # activation for the completed gate pair
                        zt = zb[p][g_pair]
                        gv = zt[0:HH, :].rearrange("p (h x) -> p h x", h=2)[:, :, tl * 8:tl * 8 + 8]
                        if gates == "f":
                            sf = gpool.tile([HH, 2, NSEQ], F32, tag="sf")
                            nc.scalar.activation(sf[:], gv, AF.Sigmoid)
                        elif gates == "i":
                            si = gpool.tile([HH, 2, NSEQ], F32, tag="si")
                            nc.scalar.activation(si[:], gv, AF.Sigmoid)
                            # c *= sf  (may start while j/o matmuls run)
                            nc.vector.tensor_mul(c_s[:], c_s[:], sf[:])
                        elif gates == "j":
                            tj = gpool.tile([HH, 2, NSEQ], F32, tag="tj")
                            nc.scalar.activation(tj[:], gv, AF.Tanh)
                            t1 = gpool.tile([HH, 2, NSEQ], F32, tag="t1")
                            nc.vector.tensor_mul(t1[:], si[:], tj[:])
                            nc.vector.tensor_add(c_s[:], c_s[:], t1[:])
                        else:
                            so = gpool.tile([HH, 2, NSEQ], F32, tag="so")
                            nc.scalar.activation(so[:], gv, AF.Sigmoid)
                            tch = gpool.tile([HH, 2, NSEQ], F32, tag="tch")
                            nc.scalar.activation(tch[:], c_s[:], AF.Tanh)
                            nc.vector.tensor_mul(h_seq[:, t], tch[:], so[:])
                    if zx_extra is not None:
                        emit_zx_mm(*zx_extra)

                for i in range(24):
                    emit_zx_mm(0, i)
                if debug_z:
                    emit_step(0, 0, None)
                    for j in range(4):
                        db = gpool.tile([128, 512], F32, tag=f"zd{j}", name=f"zd{j}")
                        nc.vector.tensor_copy(db[:], zb[0][j][:])
                        nc.sync.dma_start(zdbg_d[:, j * 512:(j + 1) * 512], db[:])
                else:
                    for c in range(n_chunks):
                        for tl in range(SCAN_CHUNK):
                            zx_extra = None
                            if c + 1 < n_chunks and tl < 24:
                                zx_extra = (c + 1, tl)
                            emit_step(c, tl, zx_extra)

            # ================= decode =================
            if debug_z:
                decode_ranges = []
            else:
                decode_ranges = list(sub_ranges(N))
            with (
                tc.tile_pool(name="dps", bufs=2, space="PSUM") as dpool,
                tc.tile_pool(name="dsb", bufs=2) as dsb,
            ):
                hv = h_seq[:]  # [HH, T, 2, NSEQ]
                for d0, dw in decode_ranges:
                    t0 = d0 // NSEQ
                    tn = dw // NSEQ
                    ps = dpool.tile([128, 512], F32, tag="dps")
                    for ki in range(2):
                        nc.tensor.matmul(ps[0:5, 0:dw], wd_s[:, ki * 8:ki * 8 + 5],
                                         hv[:, t0:t0 + tn, ki, :],
                                         start=(ki == 0), stop=(ki == 1))
                    sb = dsb.tile([5, 512], F32, tag="dsb")
                    nc.vector.tensor_copy(sb[:, 0:dw], ps[0:5, 0:dw])
                    nc.sync.dma_start(out_d[:, d0:d0 + dw], sb[:, 0:dw])

    nc.finalize()
    return nc


# ====================== host packing ======================

def _pack_weights(k1w, k1aw, k1ab, k2w, k3w, W, b, Wd_half, flip):
    """Pack one core's weights into the kernel's DRAM layouts."""
    if flip:
        k1w = k1w[::-1]
        k1aw = k1aw[::-1]
        k2w = k2w[::-1]
        k3w = k3w[::-1]
    out = {}
    out["k1w"] = np.ascontiguousarray(k1w[0, 0][None, :]).astype(BF16)
    out["k1aw"] = np.ascontiguousarray(k1aw[0, 0][None, :]).astype(BF16)
    k1ab_p = np.zeros((128, 2), np.float32)
    k1ab_p[:, 0] = k1ab[0:128]
    k1ab_p[:, 1] = k1ab[128:256]
    out["k1ab"] = k1ab_p
    k2p = np.zeros((128, 12 * 128), BF16)
    for s in range(3):
        for ki in range(2):
            for mc in range(2):
                idx = (s * 2 + ki) * 2 + mc
                k2p[:, idx * 128:(idx + 1) * 128] = k2w[s, ki * 128:(ki + 1) * 128,
                                                        mc * 128:(mc + 1) * 128].astype(BF16)
    out["k2w"] = k2p
    k3p = np.zeros((128, 4 * 128), BF16)
    for ki in range(2):
        for mc in range(2):
            idx = ki * 2 + mc
            k3p[:, idx * 128:(idx + 1) * 128] = k3w[0, ki * 128:(ki + 1) * 128,
                                                    mc * 128:(mc + 1) * 128].astype(BF16)
    out["k3w"] = k3p
    Wx = W[0:C]          # [256, 800]
    Whr = W[C:C + H]     # [200, 800]
    wxp = np.zeros((128, 16 * 128), BF16)
    whp = np.zeros((HH, 16 * 128), BF16)
    for ki in range(2):
        for g in range(8):
            idx = ki * 8 + g
            wxp[:, idx * 128:idx * 128 + HH] = Wx[ki * 128:(ki + 1) * 128,
                                                  g * HH:(g + 1) * HH].astype(BF16)
            whp[:, idx * 128:idx * 128 + HH] = Whr[ki * HH:(ki + 1) * HH,
                                                   g * HH:(g + 1) * HH].astype(BF16)
    out["wx"] = wxp
    out["whr"] = whp
    bz = b.astype(np.float32).copy()
    bz[2 * H:3 * H] += 1.0            # forget-gate bias
    bzp = np.zeros((1, 8 * 128), BF16)
    for g in range(8):
        bzp[0, g * 128:g * 128 + HH] = bz[g * HH:(g + 1) * HH].astype(BF16)
    out["bz"] = bzp
    wdp = np.zeros((HH, 16), BF16)
    for ki in range(2):
        wdp[:, ki * 8:ki * 8 + 5] = Wd_half[ki * HH:(ki + 1) * HH].astype(BF16)
    out["wd"] = wdp
    return out


def pack_core_inputs(signals, lengths, k1w, k1aw, k1ab, k2w, k3w,
                     Wf, bf, Wb, bb, Wd, T=T_FULL):
    """Build in_maps for the 8 cores: [g0fw, g0bw, g1fw, g1bw, ...]."""
    fw_w = _pack_weights(k1w, k1aw, k1ab, k2w, k3w, Wf, bf, Wd[0:H], flip=False)
    bw_w = _pack_weights(k1w, k1aw, k1ab, k2w, k3w, Wb, bb, Wd[H:2 * H], flip=True)
    in_maps = []
    for g in range(4):
        sl = slice(g * NSEQ, (g + 1) * NSEQ)
        sig_g = signals[sl, :, 0]            # [8, T]
        L_g = lengths[sl]
        sig_fw = np.zeros((1, (T + 2) * NSEQ), np.float32)
        sig_fw[0, NSEQ:NSEQ + T * NSEQ] = sig_g.T.reshape(-1)
        y = np.zeros((NSEQ, T + 2), np.float32)   # y[:, 0] = t=-1 halo
        for b_ in range(NSEQ):
            L = int(L_g[b_])
            y[b_, 1:1 + L] = sig_g[b_, L - 1::-1]
            if L < T:
                y[b_, 0] = sig_g[b_, L]           # x[L] enters x_rev[0]'s conv
        sig_bw = np.ascontiguousarray(y.T).reshape(1, (T + 2) * NSEQ)
        in_maps.append({"sig": sig_fw.astype(BF16), **fw_w})
        in_maps.append({"sig": sig_bw.astype(BF16), **bw_w})
    return in_maps


def unpack_results(results, lengths, bd, T=T_FULL):
    """Combine per-core partial logits into [B, T, 5] fp32."""
    logits = np.zeros((B, T, 5), np.float32)
    t_idx = np.arange(T)
    for g in range(4):
        p_fw = results[2 * g]["logits"].reshape(5, T, NSEQ)
        p_bw = results[2 * g + 1]["logits"].reshape(5, T, NSEQ)
        for b_ in range(NSEQ):
            L = int(lengths[g * NSEQ + b_])
            row = np.zeros((T, 5), np.float32)
            row[:L] = p_fw[:, :L, b_].T
            row[:L] += p_bw[:, L - 1::-1, b_].T[:L] if L > 0 else 0
            logits[g * NSEQ + b_] = row + bd[None, :]
    return logits


_NC_CACHE = {}


def _get_program(T=T_FULL):
    if T not in _NC_CACHE:
        _NC_CACHE[T] = build_program(T)
    return _NC_CACHE[T]


def kernel(signals, sig_length, k1w, k1aw, k1ab, k2w, k3w, Wf, bf, Wb, bb, Wd, bd):
    from concourse.bass_utils import run_bass_kernel_spmd

    signals = np.asarray(signals, np.float32)
    lengths = np.asarray(sig_length)
    nc = _get_program(T_FULL)
    in_maps = pack_core_inputs(signals, lengths,
                               np.asarray(k1w, np.float32), np.asarray(k1aw, np.float32),
                               np.asarray(k1ab, np.float32), np.asarray(k2w, np.float32),
                               np.asarray(k3w, np.float32), np.asarray(Wf, np.float32),
                               np.asarray(bf, np.float32), np.asarray(Wb, np.float32),
                               np.asarray(bb, np.float32), np.asarray(Wd, np.float32))
    res = run_bass_kernel_spmd(nc, in_maps, list(range(N_CORES)))
    return unpack_results(res.results, lengths, np.asarray(bd, np.float32)).astype(np.float32)


# ================= numpy shard model (for validation) =================

def shard_reference_np(in_map, T=T_FULL):
    """Pure-numpy emulation of one core's program from its packed inputs."""
    f32 = np.float32
    sig_full = in_map["sig"].astype(f32).reshape(T + 2, NSEQ)  # halo rows 0, T+1
    sig = sig_full[1:T + 1]                                   # [t, b]
    halo_lo, halo_hi = sig_full[0], sig_full[T + 1]
    k1w = in_map["k1w"].astype(f32)[0]                        # [256]
    k1aw = in_map["k1aw"].astype(f32)[0]
    k1ab_p = in_map["k1ab"].astype(f32)
    k1ab = np.concatenate([k1ab_p[:, 0], k1ab_p[:, 1]])
    k2p = in_map["k2w"].astype(f32)
    k2w = np.zeros((3, C, C), f32)
    for s in range(3):
        for ki in range(2):
            for mc in range(2):
                idx = (s * 2 + ki) * 2 + mc
                k2w[s, ki * 128:(ki + 1) * 128, mc * 128:(mc + 1) * 128] = \
                    k2p[:, idx * 128:(idx + 1) * 128]
    k3p = in_map["k3w"].astype(f32)
    k3w = np.zeros((C, C), f32)
    for ki in range(2):
        for mc in range(2):
            idx = ki * 2 + mc
            k3w[ki * 128:(ki + 1) * 128, mc * 128:(mc + 1) * 128] = \
                k3p[:, idx * 128:(idx + 1) * 128]
    wxp = in_map["wx"].astype(f32)
    Wx = np.zeros((C, G4), f32)
    whp = in_map["whr"].astype(f32)
    Whr = np.zeros((H, G4), f32)
    for ki in range(2):
        for g in range(8):
            idx = ki * 8 + g
            Wx[ki * 128:(ki + 1) * 128, g * HH:(g + 1) * HH] = wxp[:, idx * 128:idx * 128 + HH]
            Whr[ki * HH:(ki + 1) * HH, g * HH:(g + 1) * HH] = whp[:, idx * 128:idx * 128 + HH]
    bzp = in_map["bz"].astype(f32)
    bz = np.zeros(G4, f32)
    for g in range(8):
        bz[g * HH:(g + 1) * HH] = bzp[0, g * 128:g * 128 + HH]
    wdp = in_map["wd"].astype(f32)
    Wd_half = np.zeros((H, 5), f32)
    for ki in range(2):
        Wd_half[ki * HH:(ki + 1) * HH] = wdp[:, ki * 8:ki * 8 + 5]

    s3_ext = sig_full[:, :, None]                             # [T+2, b, 1]
    c1_ext = np.maximum(s3_ext * k1w[None, None, :], 0)       # halo rows included
    c1a = np.maximum(sig[:, :, None] * k1aw[None, None, :] + k1ab, 0)
    c2 = np.zeros((T, NSEQ, C), f32)
    for s in range(3):
        c2 += c1_ext[s:s + T] @ k2w[s]                        # valid conv on ext
    c2 = np.maximum(c2, 0)
    c3 = np.maximum(c2 @ k3w, 0)
    enc = (c3 + c1a).astype(BF16).astype(f32)                 # [t, b, 256]

    def sigmoid(x):
        return 1.0 / (1.0 + np.exp(-x))

    h = np.zeros((NSEQ, H), f32)
    c = np.zeros((NSEQ, H), f32)
    hs = np.zeros((T, NSEQ, H), f32)
    for t in range(T):
        z = enc[t] @ Wx + h @ Whr + bz
        i, j, fo, o = z[:, :H], z[:, H:2 * H], z[:, 2 * H:3 * H], z[:, 3 * H:]
        c = c * sigmoid(fo) + sigmoid(i) * np.tanh(j)
        h = (np.tanh(c) * sigmoid(o)).astype(BF16).astype(f32)
        hs[t] = h
    logits = hs @ Wd_half                                     # [t, b, 5]
    return np.ascontiguousarray(logits.transpose(2, 0, 1)).reshape(5, T * NSEQ)


if __name__ == "__main__":
    import reference
    inputs = {k: np.asarray(v) for k, v in reference.setup_inputs().items()}
    expected = np.asarray(reference.reference(**inputs))
    actual = kernel(**inputs)
    err = np.abs(actual - expected).max() / (np.abs(expected).max() + 1e-9)
    print("Relative error:", err)


# revision 35
# speedup vs baseline: 1.2103x; 1.2103x over previous
"""Bidirectional-LSTM basecaller kernel for 8 Trainium2 NeuronCores (Bass/Tile).

Model: conv front-end (1x1 -> 256, relu; 3x1 256->256, relu; 1x1 256->256,
relu; + 1x1 skip w/ bias) -> LSTM(H=200) per direction -> dense 2H->5.
B=32, T=2048.

Sharding: 4 batch groups x 2 directions = 8 cores, 8 sequences per core,
same SPMD program everywhere. The backward direction is realized host-side
by left-aligned per-sequence time reversal with a zero tail plus flipped
conv kernels, which makes the on-device scan completely mask-free; the host
un-reverses the tiny per-core partial logits and applies the length mask.

On-device layout (per core):
  - conv runs in channel-transposed layout [256ch -> 2x128 partitions,
    (t,b) free, t-major], K=3 conv = 6 accumulating matmuls with +-8-column
    shifted moving operands.
  - z_x = enc @ Wx + b is precomputed into PSUM per 32-step chunk (4 banks
    per parity, ping-pong); bias enters via a K=1 matmul against ones.
  - scan step: 16 matmuls (8 gate-tiles of 100 x 2 K-halves of 100, Wh
    stationary bf16) accumulate the recurrent term onto z_x in PSUM;
    sigmoid/tanh run on [100,2,8] PSUM slices; state c/h kept [100,(2,8)]
    so the new h is directly the next step's moving operand. No transposes.
  - decode tail: h_seq @ Wd_half -> partial logits [5, T*8] fp32 -> HBM.
"""
import sys
import numpy as np

sys.path.insert(0, "/opt/trn_rl_repo")

import ml_dtypes

BF16 = ml_dtypes.bfloat16

B, T_FULL, H, C = 32, 2048, 200, 256
NSEQ = 8          # sequences per core
HH = 100          # half hidden
G4 = 4 * H        # 800
N_CORES = 8
SCAN_CHUNK = 16   # timesteps per PSUM parity chunk

# On-device gate-tile order along the 800 axis: [i0,i1,f0,f1,o0,o1,j0,j1]
# (sigmoid gates first so one ACT covers banks 0-2, tanh j in bank 3).
# Entry n gives the source 100-column block of the reference [i|j|f|o] layout.
GATE_BLOCKS = [0, 1, 4, 5, 6, 7, 2, 3]


def build_program(T=T_FULL, conv_chunk=256, debug_z=False):
    """Emit the SPMD per-core program. Returns finalized Bacc object."""
    import concourse.bass as bass
    import concourse.tile as tile
    from concourse import bacc, mybir

    dt = mybir.dt
    BF = dt.bfloat16
    F32 = dt.float32
    AF = mybir.ActivationFunctionType

    assert T % 64 == 0
    N = T * NSEQ
    CC = min(conv_chunk, T)          # conv chunk timesteps
    NCC = CC * NSEQ                  # conv chunk cols
    n_conv_chunks = T // CC
    n_chunks = T // SCAN_CHUNK       # scan chunks

    nc = bacc.Bacc()

    # cols 0:8 = t=-1 halo, 8:8+N = data, 8+N:16+N = t=T halo (host-filled)
    sig_d = nc.declare_dram_parameter("sig", [1, N + 16], BF, isOutput=False)
    k1w_d = nc.declare_dram_parameter("k1w", [1, C], BF, isOutput=False)
    k1aw_d = nc.declare_dram_parameter("k1aw", [1, C], BF, isOutput=False)
    k1ab_d = nc.declare_dram_parameter("k1ab", [128, 2], F32, isOutput=False)
    k2w_d = nc.declare_dram_parameter("k2w", [128, 12 * 128], BF, isOutput=False)
    k3w_d = nc.declare_dram_parameter("k3w", [128, 4 * 128], BF, isOutput=False)
    wx_d = nc.declare_dram_parameter("wx", [128, 16 * 128], BF, isOutput=False)
    whr_d = nc.declare_dram_parameter("whr", [128, 16 * 128], BF, isOutput=False)
    bz_d = nc.declare_dram_parameter("bz", [1, 8 * 128], BF, isOutput=False)
    wd_d = nc.declare_dram_parameter("wd", [HH, 16], BF, isOutput=False)
    out_d = nc.declare_dram_parameter("logits", [5, N], F32, isOutput=True)
    zdbg_d = None
    if debug_z:
        zdbg_d = nc.declare_dram_parameter("zdbg", [128, 2 * 512], F32, isOutput=True)

    with tile.TileContext(nc) as tc:
        with (
            tc.tile_pool(name="wpool", bufs=1) as wpool,
            tc.tile_pool(name="state", bufs=1) as spool,
        ):
            # ---- persistent tiles ----
            k1w_s = wpool.tile([1, C], BF, tag="k1w")
            k1aw_s = wpool.tile([1, C], BF, tag="k1aw")
            k1ab_s = wpool.tile([128, 2], F32, tag="k1ab")
            k2w_s = wpool.tile([128, 12 * 128], BF, tag="k2w")
            k3w_s = wpool.tile([128, 4 * 128], BF, tag="k3w")
            wx_s = wpool.tile([128, 16 * 128], BF, tag="wx")
            whr_s = wpool.tile([128, 16 * 128], BF, tag="whr")
            bz_s = wpool.tile([1, 8 * 128], BF, tag="bz")
            wd_s = wpool.tile([HH, 16], BF, tag="wd")
            ones_s = wpool.tile([1, 256], BF, tag="ones")
            encT = [wpool.tile([128, N], BF, tag=f"encT{mc}", name=f"encT{mc}") for mc in range(2)]
            h_seq = wpool.tile([128, T, 2, NSEQ], BF, tag="h_seq")
            h0_s = spool.tile([128, 2, NSEQ], BF, tag="h0")
            c_s = spool.tile([HH, 2, NSEQ], F32, tag="c")

            nc.sync.dma_start(k1w_s[:], k1w_d[:])
            nc.sync.dma_start(k1aw_s[:], k1aw_d[:])
            nc.sync.dma_start(k1ab_s[:], k1ab_d[:])
            nc.sync.dma_start(k2w_s[:], k2w_d[:])
            nc.sync.dma_start(k3w_s[:], k3w_d[:])
            nc.sync.dma_start(wx_s[:], wx_d[:])
            nc.sync.dma_start(whr_s[:], whr_d[:])
            nc.sync.dma_start(bz_s[:], bz_d[:])
            nc.sync.dma_start(wd_s[:], wd_d[:])
            nc.vector.memset(ones_s[:], 1.0)
            nc.vector.memset(h0_s[:], 0.0)
            nc.vector.memset(c_s[:], 0.0)
            # zero the K-pad rows (base partition must be 32-aligned; rows
            # 96:100 are re-written by every step's h update before any read)
            nc.vector.memset(h_seq[96:128], 0.0)

            # ================= conv front-end (paced generators) ==========
            def sub_ranges(total, step=512):
                off = 0
                while off < total:
                    w = min(step, total - off)
                    yield off, w
                    off += w

            with (
                tc.tile_pool(name="convsb", bufs=2) as cvp,
                tc.tile_pool(name="convps", bufs=4, space="PSUM") as cps,
                tc.tile_pool(name="zps", bufs=1, space="PSUM") as zpool,
                tc.tile_pool(name="gates", bufs=2) as gpool,
                tc.tile_pool(name="dsb", bufs=2) as dsb,
            ):
                def conv_chunk_gen(cc):
                    """Generator emitting one conv chunk; each yield is a
                    pacing quantum interleaved into the scan."""
                    n0 = cc * NCC
                    sgt = cvp.tile([1, NCC + 16], BF, tag="sigc")
                    nc.sync.dma_start(sgt[:], sig_d[0:1, n0:n0 + NCC + 16])
                    yield
                    c1t = [cvp.tile([128, NCC + 16], BF, tag=f"c1t{mc}", name=f"c1t{mc}")
                           for mc in range(2)]
                    c2t = [cvp.tile([128, NCC], BF, tag=f"c2t{mc}", name=f"c2t{mc}")
                           for mc in range(2)]
                    # conv1 (with halo cols)
                    for mc in range(2):
                        for off, w in sub_ranges(NCC + 16):
                            ps = cps.tile([128, 512], F32, tag="cps")
                            nc.tensor.matmul(ps[:, 0:w], k1w_s[0:1, mc * 128:(mc + 1) * 128],
                                             sgt[0:1, off:off + w], start=True, stop=True)
                            yield
                            nc.scalar.activation(c1t[mc][:, off:off + w], ps[:, 0:w], AF.Relu)
                            yield
                    # conv1a -> encT (with per-channel bias)
                    for mc in range(2):
                        for off, w in sub_ranges(NCC):
                            ps = cps.tile([128, 512], F32, tag="cps")
                            nc.tensor.matmul(ps[:, 0:w], k1aw_s[0:1, mc * 128:(mc + 1) * 128],
                                             sgt[0:1, 8 + off:8 + off + w], start=True, stop=True)
                            yield
                            nc.scalar.activation(encT[mc][:, n0 + off:n0 + off + w], ps[:, 0:w],
                                                 AF.Relu, bias=k1ab_s[:, mc:mc + 1])
                            yield
                    # conv2: 3 shifts x 2 cin tiles accumulated
                    for mc in range(2):
                        for off, w in sub_ranges(NCC):
                            ps = cps.tile([128, 512], F32, tag="cps")
                            first = True
                            for s in range(3):
                                for ki in range(2):
                                    idx = (s * 2 + ki) * 2 + mc
                                    nc.tensor.matmul(
                                        ps[:, 0:w], k2w_s[:, idx * 128:(idx + 1) * 128],
                                        c1t[ki][:, off + s * 8:off + s * 8 + w],
                                        start=first, stop=(s == 2 and ki == 1))
                                    first = False
                                    yield
                            nc.scalar.activation(c2t[mc][:, off:off + w], ps[:, 0:w], AF.Relu)
                            yield
                    # conv3 + skip add into encT
                    for mc in range(2):
                        for off, w in sub_ranges(NCC):
                            ps = cps.tile([128, 512], F32, tag="cps")
                            for ki in range(2):
                                idx = ki * 2 + mc
                                nc.tensor.matmul(ps[:, 0:w], k3w_s[:, idx * 128:(idx + 1) * 128],
                                                 c2t[ki][:, off:off + w],
                                                 start=(ki == 0), stop=(ki == 1))
                                yield
                            tmp = cvp.tile([128, 512], BF, tag="tmp")
                            nc.scalar.activation(tmp[:, 0:w], ps[:, 0:w], AF.Relu)
                            yield
                            nc.vector.tensor_add(encT[mc][:, n0 + off:n0 + off + w],
                                                 tmp[:, 0:w], encT[mc][:, n0 + off:n0 + off + w])
                            yield

                convgens = [conv_chunk_gen(cc) for cc in range(n_conv_chunks)]
                conv_done = [False] * n_conv_chunks

                def drain_conv(up_to):
                    for cc in range(min(up_to + 1, n_conv_chunks)):
                        if not conv_done[cc]:
                            for _ in convgens[cc]:
                                pass
                            conv_done[cc] = True

                def advance_conv(k):
                    for _ in range(k):
                        for cc in range(n_conv_chunks):
                            if not conv_done[cc]:
                                try:
                                    next(convgens[cc])
                                except StopIteration:
                                    conv_done[cc] = True
                                    continue
                                break
                        else:
                            return

                # ================= LSTM scan =================
                # One 2-bank PSUM tensor per parity: bank b holds gate-tiles
                # 4b..4b+3 (order i,i,f,f | o,o,j,j) in 128-col quarters.
                zbt = [zpool.tile([128, 2, 512], F32, tag=f"zp{p}", name=f"zp{p}")
                       for p in range(2)]
                SC = SCAN_CHUNK
                NSC = SC * NSEQ  # cols per scan chunk

                def emit_zx_mm(c, i):
                    """i-th of the 24 z_x matmuls for scan chunk c. PSUM
                    start=True zeroes the WHOLE bank, so only the first matmul
                    touching each bank per chunk sets it; later first-touches
                    inherit overwrite-on-first-touch from pending-zero bytes."""
                    p = c & 1
                    g, which = divmod(i, 3)
                    b, q = divmod(g, 4)
                    out = zbt[p][:, b, q * 128:(q + 1) * 128]
                    if which < 2:
                        ki = which
                        nc.tensor.matmul(
                            out, wx_s[:, (ki * 8 + g) * 128:(ki * 8 + g + 1) * 128],
                            encT[ki][:, c * NSC:(c + 1) * NSC],
                            start=(which == 0 and q == 0), stop=False,
                            skip_group_check=True)
                    else:
                        nc.tensor.matmul(
                            out, bz_s[0:1, g * 128:(g + 1) * 128], ones_s[0:1, 0:NSC],
                            start=False, stop=False, skip_group_check=True)

                def emit_step(c, tl, zx_ids):
                    p = c & 1
                    t = c * SC + tl
                    hprev = h0_s if t == 0 else h_seq[:, t - 1]
                    zt = zbt[p]
                    for g in range(8):
                        b, q = divmod(g, 4)
                        out = zt[:, b, q * 128 + tl * 8:q * 128 + tl * 8 + 8]
                        for ki in range(2):
                            nc.tensor.matmul(
                                out, whr_s[:, (ki * 8 + g) * 128:(ki * 8 + g + 1) * 128],
                                hprev[:, ki],
                                start=False, stop=(ki == 1), skip_group_check=True)
                    # One sigmoid over all four gates; j weights doubled
                    # host-side so tanh(zj) = 2*sigmoid(2 zj) - 1.
                    zv = zt[0:HH].rearrange("p b (q x) -> p b q x", q=4)
                    sg = gpool.tile([HH, 2, 4, NSEQ], F32, tag="sg")
                    nc.scalar.activation(sg[:], zv[:, :, :, tl * 8:tl * 8 + 8], AF.Sigmoid)
                    si, sf = sg[:, 0, 0:2], sg[:, 0, 2:4]
                    so, sj = sg[:, 1, 0:2], sg[:, 1, 2:4]
                    # c = c*sf + si*(2*sg_j - 1) ; h = tanh(c)*so
                    nc.gpsimd.tensor_mul(c_s[:], c_s[:], sf)
                    u = gpool.tile([HH, 2, NSEQ], F32, tag="u")
                    nc.vector.scalar_tensor_tensor(
                        u[:], sj, -0.5, si,
                        op0=mybir.AluOpType.add, op1=mybir.AluOpType.mult)
                    nc.vector.scalar_tensor_tensor(
                        c_s[:], u[:], 2.0, c_s[:],
                        op0=mybir.AluOpType.mult, op1=mybir.AluOpType.add)
                    tch = gpool.tile([HH, 2, NSEQ], F32, tag="tch")
                    nc.scalar.activation(tch[:], c_s[:], AF.Tanh)
                    nc.vector.tensor_mul(h_seq[0:HH, t], tch[:], so)
                    for i in zx_ids:
                        emit_zx_mm(c + 1, i)

                def emit_decode(t0):
                    """Partial logits for t in [t0, t0+64) -> HBM."""
                    ps = cps.tile([128, 512], F32, tag="cps")
                    hv = h_seq[0:HH]
                    for ki in range(2):
                        nc.tensor.matmul(ps[0:5, :], wd_s[:, ki * 8:ki * 8 + 5],
                                         hv[:, t0:t0 + 64, ki, :],
                                         start=(ki == 0), stop=(ki == 1))
                    sb = dsb.tile([5, 512], F32, tag="dsb")
                    nc.vector.tensor_copy(sb[:], ps[0:5, :])
                    nc.sync.dma_start(out_d[:, t0 * NSEQ:t0 * NSEQ + 512], sb[:])

                drain_conv(0)
                for i in range(24):
                    emit_zx_mm(0, i)
                if debug_z:
                    emit_step(0, 0, [])
                    for j in range(2):
                        db = gpool.tile([128, 512], F32, tag=f"zd{j}", name=f"zd{j}")
                        nc.vector.tensor_copy(db[:], zbt[0][:, j, :])
                        nc.sync.dma_start(zdbg_d[:, j * 512:(j + 1) * 512], db[:])
                else:
                    dec_next = 0
                    for c in range(n_chunks):
                        # conv chunk feeding zx(c+1) must be fully emitted
                        rc = ((c + 2) * NSC - 1) // NCC
                        drain_conv(rc)
                        for tl in range(SC):
                            if c + 1 < n_chunks:
                                zx_ids = range((tl * 24) // SC, ((tl + 1) * 24) // SC)
                            else:
                                zx_ids = []
                            emit_step(c, tl, zx_ids)
                            advance_conv(2)
                        while dec_next + 64 <= (c + 1) * SC:
                            emit_decode(dec_next)
                            dec_next += 64
                    while dec_next + 64 <= T:
                        emit_decode(dec_next)
                        dec_next += 64

    nc.finalize()
    return nc


# ====================== host packing ======================

def _pack_weights(k1w, k1aw, k1ab, k2w, k3w, W, b, Wd_half, flip):
    """Pack one core's weights into the kernel's DRAM layouts."""
    if flip:
        k1w = k1w[::-1]
        k1aw = k1aw[::-1]
        k2w = k2w[::-1]
        k3w = k3w[::-1]
    out = {}
    out["k1w"] = np.ascontiguousarray(k1w[0, 0][None, :]).astype(BF16)
    out["k1aw"] = np.ascontiguousarray(k1aw[0, 0][None, :]).astype(BF16)
    k1ab_p = np.zeros((128, 2), np.float32)
    k1ab_p[:, 0] = k1ab[0:128]
    k1ab_p[:, 1] = k1ab[128:256]
    out["k1ab"] = k1ab_p
    k2p = np.zeros((128, 12 * 128), BF16)
    for s in range(3):
        for ki in range(2):
            for mc in range(2):
                idx = (s * 2 + ki) * 2 + mc
                k2p[:, idx * 128:(idx + 1) * 128] = k2w[s, ki * 128:(ki + 1) * 128,
                                                        mc * 128:(mc + 1) * 128].astype(BF16)
    out["k2w"] = k2p
    k3p = np.zeros((128, 4 * 128), BF16)
    for ki in range(2):
        for mc in range(2):
            idx = ki * 2 + mc
            k3p[:, idx * 128:(idx + 1) * 128] = k3w[0, ki * 128:(ki + 1) * 128,
                                                    mc * 128:(mc + 1) * 128].astype(BF16)
    out["k3w"] = k3p
    Wx = W[0:C]          # [256, 800]
    Whr = W[C:C + H]     # [200, 800]
    wxp = np.zeros((128, 16 * 128), BF16)
    whp = np.zeros((128, 16 * 128), BF16)   # K padded 100->128 (zeros)
    bz = b.astype(np.float32).copy()
    bz[2 * H:3 * H] += 1.0            # forget-gate bias
    bzp = np.zeros((1, 8 * 128), BF16)
    for g, blk in enumerate(GATE_BLOCKS):
        # j-gate (device tiles 6,7) weights doubled: tanh(x) = 2*sigmoid(2x)-1
        sc = 2.0 if g >= 6 else 1.0
        for ki in range(2):
            idx = ki * 8 + g
            wxp[:, idx * 128:idx * 128 + HH] = (sc * Wx[ki * 128:(ki + 1) * 128,
                                                blk * HH:(blk + 1) * HH]).astype(BF16)
            whp[0:HH, idx * 128:idx * 128 + HH] = (sc * Whr[ki * HH:(ki + 1) * HH,
                                                   blk * HH:(blk + 1) * HH]).astype(BF16)
        bzp[0, g * 128:g * 128 + HH] = (sc * bz[blk * HH:(blk + 1) * HH]).astype(BF16)
    out["wx"] = wxp
    out["whr"] = whp
    out["bz"] = bzp
    wdp = np.zeros((HH, 16), BF16)
    for ki in range(2):
        wdp[:, ki * 8:ki * 8 + 5] = Wd_half[ki * HH:(ki + 1) * HH].astype(BF16)
    out["wd"] = wdp
    return out


def pack_core_inputs(signals, lengths, k1w, k1aw, k1ab, k2w, k3w,
                     Wf, bf, Wb, bb, Wd, T=T_FULL):
    """Build in_maps for the 8 cores: [g0fw, g0bw, g1fw, g1bw, ...]."""
    fw_w = _pack_weights(k1w, k1aw, k1ab, k2w, k3w, Wf, bf, Wd[0:H], flip=False)
    bw_w = _pack_weights(k1w, k1aw, k1ab, k2w, k3w, Wb, bb, Wd[H:2 * H], flip=True)
    in_maps = []
    for g in range(4):
        sl = slice(g * NSEQ, (g + 1) * NSEQ)
        sig_g = signals[sl, :, 0]            # [8, T]
        L_g = lengths[sl]
        sig_fw = np.zeros((1, (T + 2) * NSEQ), np.float32)
        sig_fw[0, NSEQ:NSEQ + T * NSEQ] = sig_g.T.reshape(-1)
        y = np.zeros((NSEQ, T + 2), np.float32)   # y[:, 0] = t=-1 halo
        for b_ in range(NSEQ):
            L = int(L_g[b_])
            y[b_, 1:1 + L] = sig_g[b_, L - 1::-1]
            if L < T:
                y[b_, 0] = sig_g[b_, L]           # x[L] enters x_rev[0]'s conv
        sig_bw = np.ascontiguousarray(y.T).reshape(1, (T + 2) * NSEQ)
        in_maps.append({"sig": sig_fw.astype(BF16), **fw_w})
        in_maps.append({"sig": sig_bw.astype(BF16), **bw_w})
    return in_maps


def unpack_results(results, lengths, bd, T=T_FULL):
    """Combine per-core partial logits into [B, T, 5] fp32."""
    logits = np.zeros((B, T, 5), np.float32)
    t_idx = np.arange(T)
    for g in range(4):
        p_fw = results[2 * g]["logits"].reshape(5, T, NSEQ)
        p_bw = results[2 * g + 1]["logits"].reshape(5, T, NSEQ)
        for b_ in range(NSEQ):
            L = int(lengths[g * NSEQ + b_])
            row = np.zeros((T, 5), np.float32)
            row[:L] = p_fw[:, :L, b_].T
            row[:L] += p_bw[:, L - 1::-1, b_].T[:L] if L > 0 else 0
            logits[g * NSEQ + b_] = row + bd[None, :]
    return logits


_NC_CACHE = {}


def _get_program(T=T_FULL):
    if T not in _NC_CACHE:
        _NC_CACHE[T] = build_program(T)
    return _NC_CACHE[T]


def kernel(signals, sig_length, k1w, k1aw, k1ab, k2w, k3w, Wf, bf, Wb, bb, Wd, bd):
    from concourse.bass_utils import run_bass_kernel_spmd

    signals = np.asarray(signals, np.float32)
    lengths = np.asarray(sig_length)
    nc = _get_program(T_FULL)
    in_maps = pack_core_inputs(signals, lengths,
                               np.asarray(k1w, np.float32), np.asarray(k1aw, np.float32),
                               np.asarray(k1ab, np.float32), np.asarray(k2w, np.float32),
                               np.asarray(k3w, np.float32), np.asarray(Wf, np.float32),
                               np.asarray(bf, np.float32), np.asarray(Wb, np.float32),
                               np.asarray(bb, np.float32), np.asarray(Wd, np.float32))
    res = run_bass_kernel_spmd(nc, in_maps, list(range(N_CORES)))
    return unpack_results(res.results, lengths, np.asarray(bd, np.float32)).astype(np.float32)


# ================= numpy shard model (for validation) =================

def shard_reference_np(in_map, T=T_FULL):
    """Pure-numpy emulation of one core's program from its packed inputs."""
    f32 = np.float32
    sig_full = in_map["sig"].astype(f32).reshape(T + 2, NSEQ)  # halo rows 0, T+1
    sig = sig_full[1:T + 1]                                   # [t, b]
    halo_lo, halo_hi = sig_full[0], sig_full[T + 1]
    k1w = in_map["k1w"].astype(f32)[0]                        # [256]
    k1aw = in_map["k1aw"].astype(f32)[0]
    k1ab_p = in_map["k1ab"].astype(f32)
    k1ab = np.concatenate([k1ab_p[:, 0], k1ab_p[:, 1]])
    k2p = in_map["k2w"].astype(f32)
    k2w = np.zeros((3, C, C), f32)
    for s in range(3):
        for ki in range(2):
            for mc in range(2):
                idx = (s * 2 + ki) * 2 + mc
                k2w[s, ki * 128:(ki + 1) * 128, mc * 128:(mc + 1) * 128] = \
                    k2p[:, idx * 128:(idx + 1) * 128]
    k3p = in_map["k3w"].astype(f32)
    k3w = np.zeros((C, C), f32)
    for ki in range(2):
        for mc in range(2):
            idx = ki * 2 + mc
            k3w[ki * 128:(ki + 1) * 128, mc * 128:(mc + 1) * 128] = \
                k3p[:, idx * 128:(idx + 1) * 128]
    wxp = in_map["wx"].astype(f32)
    Wx = np.zeros((C, G4), f32)
    whp = in_map["whr"].astype(f32)
    Whr = np.zeros((H, G4), f32)
    bzp = in_map["bz"].astype(f32)
    bz = np.zeros(G4, f32)
    for g, blk in enumerate(GATE_BLOCKS):
        sc = 0.5 if g >= 6 else 1.0   # undo the j-gate doubling
        for ki in range(2):
            idx = ki * 8 + g
            Wx[ki * 128:(ki + 1) * 128, blk * HH:(blk + 1) * HH] = sc * wxp[:, idx * 128:idx * 128 + HH]
            Whr[ki * HH:(ki + 1) * HH, blk * HH:(blk + 1) * HH] = sc * whp[0:HH, idx * 128:idx * 128 + HH]
        bz[blk * HH:(blk + 1) * HH] = sc * bzp[0, g * 128:g * 128 + HH]
    wdp = in_map["wd"].astype(f32)
    Wd_half = np.zeros((H, 5), f32)
    for ki in range(2):
        Wd_half[ki * HH:(ki + 1) * HH] = wdp[:, ki * 8:ki * 8 + 5]

    s3_ext = sig_full[:, :, None]                             # [T+2, b, 1]
    c1_ext = np.maximum(s3_ext * k1w[None, None, :], 0)       # halo rows included
    c1a = np.maximum(sig[:, :, None] * k1aw[None, None, :] + k1ab, 0)
    c2 = np.zeros((T, NSEQ, C), f32)
    for s in range(3):
        c2 += c1_ext[s:s + T] @ k2w[s]                        # valid conv on ext
    c2 = np.maximum(c2, 0)
    c3 = np.maximum(c2 @ k3w, 0)
    enc = (c3 + c1a).astype(BF16).astype(f32)                 # [t, b, 256]

    def sigmoid(x):
        return 1.0 / (1.0 + np.exp(-x))

    h = np.zeros((NSEQ, H), f32)
    c = np.zeros((NSEQ, H), f32)
    hs = np.zeros((T, NSEQ, H), f32)
    for t in range(T):
        z = enc[t] @ Wx + h @ Whr + bz
        i, j, fo, o = z[:, :H], z[:, H:2 * H], z[:, 2 * H:3 * H], z[:, 3 * H:]
        c = c * sigmoid(fo) + sigmoid(i) * np.tanh(j)
        h = (np.tanh(c) * sigmoid(o)).astype(BF16).astype(f32)
        hs[t] = h
    logits = hs @ Wd_half                                     # [t, b, 5]
    return np.ascontiguousarray(logits.transpose(2, 0, 1)).reshape(5, T * NSEQ)


if __name__ == "__main__":
    import reference
    inputs = {k: np.asarray(v) for k, v in reference.setup_inputs().items()}
    expected = np.asarray(reference.reference(**inputs))
    actual = kernel(**inputs)
    err = np.abs(actual - expected).max() / (np.abs(expected).max() + 1e-9)
    print("Relative error:", err)
